# revision 1
# baseline (speedup 1.0000x reference)
"""GNN NodeBlock kernel for 8 TRN2 NeuronCores.

Math: out = (segment_mean(edge_attrs @ Wp + bp, dst)) @ Wu + bu, with bp=bu=0.
Projection is linear, so segment_sum(X @ Wp) == segment_sum(X) @ Wp and the
two MLPs fuse into one 64x64 weight Wf = Wp @ Wu applied to the aggregate.

Sharding: node-parallel. The host sorts edges by destination (a pure layout /
sharding permutation) and gives each core a contiguous node range plus its
edges, grouped into tiles of <=128 consecutive nodes with a fixed edge-chunk
budget. No collectives are needed.

Device (SPMD, identical program on 8 cores):
  per 128-edge chunk:  onehot[e, r] = (iota[r] == local_dst[e])   (DVE, bf16)
                       PSUM[0:65, 0:128] += Xaug^T @ onehot       (TensorE)
                       where Xaug = [x | 1] so row 64 accumulates counts
  per tile:            S = PSUM -> SBUF (bf16), then one matmul
                       S^T @ [Wf | e_cnt] -> [128 nodes, 64 feat | counts]
                       out = mlp[:, :64] * (1 / max(counts, 1))   per-partition
Host then scatters per-tile rows back to global node ids (pure permutation).
"""

import sys

sys.path.insert(0, "/opt/trn_rl_repo")

import numpy as np

P = 128
D = 64
NCORES = 8
CAP_CHUNKS_MIN = 16  # edge-chunk budget per node tile (16*128 = 2048 slots)


def _plan_tiles(counts, cum, s, cap):
    """Greedy tiles per core: consecutive nodes, <=128 nodes, <=cap edges."""
    core_tiles = []
    for k in range(NCORES):
        a = int(s[k])
        hi = int(s[k + 1])
        tiles = []
        while a < hi:
            b_lim = min(a + P, hi)
            b = int(np.searchsorted(cum, cum[a] + cap, side="right")) - 1
            b = max(a + 1, min(b, b_lim))
            tiles.append((a, b))
            a = b
        core_tiles.append(tiles)
    return core_tiles


def _build_program(nt, cap_chunks):
    import concourse.bacc as bacc
    from concourse import mybir
    from concourse.masks import make_identity
    from concourse.tile import TileContext

    BF = mybir.dt.bfloat16
    F32 = mybir.dt.float32
    nchunk = nt * cap_chunks
    DA = D + 1  # attrs + ones column

    nc = bacc.Bacc(None, target_bir_lowering=False)
    x_d = nc.declare_dram_parameter("x", [P, nchunk * DA], BF, isOutput=False)
    ldst_d = nc.declare_dram_parameter("ldst", [P, nchunk], F32, isOutput=False)
    wp_d = nc.declare_dram_parameter("wp", [D, D], F32, isOutput=False)
    wu_d = nc.declare_dram_parameter("wu", [D, D], F32, isOutput=False)
    out_d = nc.declare_dram_parameter("out", [nt * P, D], F32, isOutput=True)

    with TileContext(nc) as tc:
        with (
            tc.tile_pool(name="const", bufs=1) as cp,
            tc.tile_pool(name="xin", bufs=4) as xp,
            tc.tile_pool(name="oh", bufs=64) as ohp,
            tc.tile_pool(name="flush", bufs=4) as fp,
            tc.tile_pool(name="small", bufs=8) as sp,
            tc.tile_pool(name="res", bufs=2) as rp,
            tc.tile_pool(name="psacc", bufs=3, space="PSUM") as psa,
            tc.tile_pool(name="psmlp", bufs=2, space="PSUM") as psb,
            tc.tile_pool(name="psw", bufs=1, space="PSUM") as psw,
        ):
            # --- constants ---
            ident = cp.tile([D, D], F32)
            make_identity(nc, ident[:])
            iota_i = cp.tile([P, P], mybir.dt.int32)
            nc.gpsimd.iota(iota_i[:], pattern=[[1, P]], base=0, channel_multiplier=0)
            iota_bf = cp.tile([P, P], BF)
            nc.vector.tensor_copy(iota_bf[:], iota_i[:])
            ldst_sb = cp.tile([P, nchunk], F32)
            nc.sync.dma_start(out=ldst_sb[:], in_=ldst_d[:])

            # --- fused weight Wf = Wp @ Wu, extended with a count column ---
            wp_sb = cp.tile([D, D], F32)
            nc.sync.dma_start(out=wp_sb[:], in_=wp_d[:])
            wu_sb = cp.tile([D, D], F32)
            nc.sync.dma_start(out=wu_sb[:], in_=wu_d[:])
            wpt_ps = psw.tile([D, D], F32)
            nc.tensor.transpose(out=wpt_ps[:], in_=wp_sb[:], identity=ident[:])
            wpt_bf = cp.tile([D, D], BF)
            nc.vector.tensor_copy(wpt_bf[:], wpt_ps[:])
            wu_bf = cp.tile([D, D], BF)
            nc.vector.tensor_copy(wu_bf[:], wu_sb[:])
            wf_ps = psw.tile([D, D], F32)
            nc.tensor.matmul(wf_ps[:], lhsT=wpt_bf[:], rhs=wu_bf[:], start=True, stop=True)
            wf_ext = cp.tile([DA, DA], BF)
            nc.gpsimd.memset(wf_ext[:], 0.0)
            nc.vector.tensor_copy(wf_ext[0:D, 0:D], wf_ps[:])
            nc.gpsimd.memset(wf_ext[D : D + 1, D : D + 1], 1.0)

            # --- main loop ---
            XB = 2  # tiles per input DMA
            OB = 4  # tiles per output DMA
            xins = {}
            outbs = {}
            for t in range(nt):
                if t % XB == 0:
                    span = min(XB, nt - t)
                    xin = xp.tile([P, XB * cap_chunks * DA], BF, tag="xin", name=f"xin{t}")
                    nc.sync.dma_start(
                        out=xin[:, : span * cap_chunks * DA],
                        in_=x_d[:, t * cap_chunks * DA : (t + span) * cap_chunks * DA],
                    )
                    xins[t] = xin
                xin = xins[t - t % XB]
                xoff = (t % XB) * cap_chunks * DA
                acc = psa.tile([DA, P], F32)
                for c in range(cap_chunks):
                    gc = t * cap_chunks + c
                    oh = ohp.tile([P, P], BF)
                    nc.vector.tensor_scalar(
                        out=oh[:],
                        in0=iota_bf[:],
                        scalar1=ldst_sb[:, gc : gc + 1],
                        scalar2=None,
                        op0=mybir.AluOpType.is_equal,
                    )
                    nc.tensor.matmul(
                        acc[:],
                        lhsT=xin[:, xoff + c * DA : xoff + (c + 1) * DA],
                        rhs=oh[:],
                        start=(c == 0),
                        stop=(c == cap_chunks - 1),
                    )
                s_bf = fp.tile([DA, P], BF)
                nc.scalar.copy(out=s_bf[:], in_=acc[:])
                mlp = psb.tile([P, DA], F32)
                nc.tensor.matmul(mlp[:], lhsT=s_bf[:], rhs=wf_ext[:], start=True, stop=True)
                cnt_m = sp.tile([P, 1], F32)
                nc.vector.tensor_scalar_max(cnt_m[:], mlp[:, D : D + 1], 1.0)
                recip = sp.tile([P, 1], F32)
                nc.vector.reciprocal(recip[:], cnt_m[:])
                if t % OB == 0:
                    outbs[t] = rp.tile([P, OB * D], F32, tag="outb", name=f"outb{t}")
                outb = outbs[t - t % OB]
                g = t % OB
                nc.scalar.activation(
                    out=outb[:, g * D : (g + 1) * D],
                    in_=mlp[:, 0:D],
                    func=mybir.ActivationFunctionType.Copy,
                    scale=recip[:, 0:1],
                )
                if t % OB == OB - 1 or t == nt - 1:
                    t0 = t - g
                    span = g + 1
                    dst_ap = out_d[t0 * P : (t0 + span) * P, :].rearrange(
                        "(g p) f -> p g f", p=P
                    )
                    nc.sync.dma_start(out=dst_ap, in_=outb[:, : span * D].rearrange("p (g f) -> p g f", f=D))

    return nc


def _prepare(inputs):
    """Host-side shard/layout prep. Returns (in_maps, core_tiles, nt, cap_chunks, N)."""
    from concourse import mybir

    bf16 = mybir.dt.np(mybir.dt.bfloat16)

    edge_attrs = np.asarray(inputs["edge_attrs"], dtype=np.float32)
    wp = np.ascontiguousarray(np.asarray(inputs["proj_W"], dtype=np.float32))
    wu = np.ascontiguousarray(np.asarray(inputs["upd_W"], dtype=np.float32))
    dst = np.asarray(inputs["dst"]).astype(np.int64).ravel()
    N = int(np.asarray(inputs["n_nodes"]))
    E = dst.shape[0]

    perm = np.argsort(dst, kind="stable")
    sdst = dst[perm]
    sx = edge_attrs[perm].astype(bf16)

    counts = np.bincount(sdst, minlength=N)
    cum = np.concatenate([[0], np.cumsum(counts)])

    # node-aligned, roughly equal-edge core split
    s = [0]
    for k in range(1, NCORES):
        s.append(int(sdst[min((k * E) // NCORES, E - 1)]))
    s.append(N)
    s = np.maximum.accumulate(np.asarray(s, dtype=np.int64))

    cap_chunks = max(CAP_CHUNKS_MIN, int(np.ceil(counts.max() / P))) if E else CAP_CHUNKS_MIN
    cap = cap_chunks * P
    core_tiles = _plan_tiles(counts, cum, s, cap)
    nt = max(len(t) for t in core_tiles)
    nchunk = nt * cap_chunks
    DA = D + 1

    in_maps = []
    for k in range(NCORES):
        x_core = np.zeros((nchunk * P, DA), dtype=bf16)
        x_core[:, D] = 1.0
        ldst_core = np.full((nchunk * P,), 1000.0, dtype=np.float32)
        for t, (a, b) in enumerate(core_tiles[k]):
            e0, e1 = int(cum[a]), int(cum[b])
            n = e1 - e0
            base = t * cap * 1
            x_core[base : base + n, :D] = sx[e0:e1]
            ldst_core[base : base + n] = (sdst[e0:e1] - a).astype(np.float32)
        x_dev = np.ascontiguousarray(
            x_core.reshape(nchunk, P, DA).transpose(1, 0, 2).reshape(P, nchunk * DA)
        )
        ldst_dev = np.ascontiguousarray(ldst_core.reshape(nchunk, P).T)
        in_maps.append({"x": x_dev, "ldst": ldst_dev, "wp": wp, "wu": wu})

    return in_maps, core_tiles, nt, cap_chunks, N


def kernel(**inputs) -> np.ndarray:
    from concourse.bass_utils import run_bass_kernel_spmd

    in_maps, core_tiles, nt, cap_chunks, N = _prepare(inputs)
    nc = _build_program(nt, cap_chunks)
    nc.finalize()
    res = run_bass_kernel_spmd(nc, in_maps, core_ids=list(range(NCORES)))

    out_full = np.zeros((N, D), dtype=np.float32)
    for k in range(NCORES):
        o = res.results[k]["out"]
        for t, (a, b) in enumerate(core_tiles[k]):
            out_full[a:b] = o[t * P : t * P + (b - a)]
    return out_full



# revision 3
# speedup vs baseline: 1.5421x; 1.5421x over previous
"""GNN NodeBlock kernel for 8 TRN2 NeuronCores.

Math: out = (segment_mean(edge_attrs @ Wp + bp, dst)) @ Wu + bu.
Projection is linear, so it commutes with the segment sum: the two MLPs fuse
into one 64x64 weight Wf = Wp @ Wu applied to the per-node aggregate, and the
mean division (a per-node row scale) commutes with the feature matmul, so it
is applied after the update MLP. Biases reduce to a constant row added on the
host (zero for this problem's inputs).

Sharding: node-parallel, equal node ranges per core. Each core's nodes are
assigned to (tile, column) slots, 128 nodes per tile. Every node gets exactly
C=16 base edge slots, so a 128-slot chunk covers 8 consecutive columns with a
FIXED aggregation pattern A16[s, n] = (s//16 == n): one constant bf16 matrix,
no per-chunk onehot work. Each chunk is one matmul with the big X chunk as the
(free-to-load) stationary operand and A16 moving: 8 PSUM columns per chunk.

Edges beyond the 16 per-node slots ("spill", ~10% of edges) go through S
data-dependent onehot chunks per tile (DVE is_equal against an iota row),
accumulated into the same PSUM tile with start=False. The host bin-packs
nodes into tiles sorted by spill count so every tile's spill fits S chunks.

Per tile: 16 fixed matmuls + S spill matmuls -> acc[64 feats, 128 nodes] in
PSUM -> Act copy to SBUF bf16 -> MLP matmul with Wf -> Act copy with
per-partition 1/count scale -> bf16 output in a [128, nt*64] column layout
(big contiguous DMA descriptors). Host scatters rows back to node ids.
"""

import sys

sys.path.insert(0, "/opt/trn_rl_repo")

import numpy as np

P = 128
D = 64
NCORES = 8
C = 16  # base edge slots per node (= edges aggregated by the fixed pattern)
CH = C  # base chunks per 128-node tile (128 nodes * C slots / 128 slot rows)
XB = 4  # tiles per base-x DMA
SXB = 8  # tiles per spill-x DMA
OB = 4  # tiles per output DMA


def _build_program(nt, S):
    import concourse.bacc as bacc
    from concourse import mybir
    from concourse.tile import TileContext

    BF = mybir.dt.bfloat16
    F32 = mybir.dt.float32
    nchunk = nt * CH
    nspill = nt * S

    nc = bacc.Bacc(None, target_bir_lowering=False)
    x_d = nc.declare_dram_parameter("x", [P, nchunk * D], BF, isOutput=False)
    sx_d = nc.declare_dram_parameter("sx", [P, nspill * D], BF, isOutput=False)
    sldst_d = nc.declare_dram_parameter("sldst", [P, nspill], F32, isOutput=False)
    wf_d = nc.declare_dram_parameter("wf", [D, D], BF, isOutput=False)
    a16_d = nc.declare_dram_parameter("a16", [P, 8], BF, isOutput=False)
    iota_d = nc.declare_dram_parameter("iota", [P, P], BF, isOutput=False)
    recip_d = nc.declare_dram_parameter("recip", [P, nt], F32, isOutput=False)
    out_d = nc.declare_dram_parameter("out", [P, nt * D], BF, isOutput=True)

    with TileContext(nc) as tc:
        with (
            tc.tile_pool(name="const", bufs=1) as cp,
            tc.tile_pool(name="xin", bufs=2) as xp,
            tc.tile_pool(name="sxin", bufs=2) as sxp,
            tc.tile_pool(name="oh", bufs=8) as ohp,
            tc.tile_pool(name="sbf", bufs=3) as fp,
            tc.tile_pool(name="res", bufs=2) as rp,
            tc.tile_pool(name="psacc", bufs=2, space="PSUM") as psa,
            tc.tile_pool(name="psmlp", bufs=2, space="PSUM") as psb,
        ):
            wf_sb = cp.tile([D, D], BF)
            nc.sync.dma_start(out=wf_sb[:], in_=wf_d[:])
            a16_sb = cp.tile([P, 8], BF)
            nc.sync.dma_start(out=a16_sb[:], in_=a16_d[:])
            iota_sb = cp.tile([P, P], BF)
            nc.sync.dma_start(out=iota_sb[:], in_=iota_d[:])
            recip_sb = cp.tile([P, nt], F32)
            nc.sync.dma_start(out=recip_sb[:], in_=recip_d[:])
            sldst_sb = cp.tile([P, nspill], F32)
            nc.sync.dma_start(out=sldst_sb[:], in_=sldst_d[:])

            xins = {}
            sxins = {}
            outbs = {}
            for t in range(nt):
                if t % XB == 0:
                    span = min(XB, nt - t)
                    xin = xp.tile([P, XB * CH * D], BF, tag="xin", name=f"xin{t}")
                    nc.sync.dma_start(
                        out=xin[:, : span * CH * D],
                        in_=x_d[:, t * CH * D : (t + span) * CH * D],
                    )
                    xins[t] = xin
                if t % SXB == 0:
                    span = min(SXB, nt - t)
                    sxin = sxp.tile([P, SXB * S * D], BF, tag="sxin", name=f"sxin{t}")
                    nc.sync.dma_start(
                        out=sxin[:, : span * S * D],
                        in_=sx_d[:, t * S * D : (t + span) * S * D],
                    )
                    sxins[t] = sxin
                xin = xins[t - t % XB]
                xoff = (t % XB) * CH * D
                sxin = sxins[t - t % SXB]
                sxoff = (t % SXB) * S * D

                acc = psa.tile([D, P], F32)
                for j in range(CH):
                    nc.tensor.matmul(
                        acc[:, 8 * j : 8 * (j + 1)],
                        lhsT=xin[:, xoff + j * D : xoff + (j + 1) * D],
                        rhs=a16_sb[:],
                        start=(j == 0),
                        stop=False,
                        skip_group_check=True,
                    )
                for s in range(S):
                    q = t * S + s
                    oh = ohp.tile([P, P], BF)
                    nc.vector.tensor_scalar(
                        out=oh[:],
                        in0=iota_sb[:],
                        scalar1=sldst_sb[:, q : q + 1],
                        scalar2=None,
                        op0=mybir.AluOpType.is_equal,
                    )
                    nc.tensor.matmul(
                        acc[:],
                        lhsT=sxin[:, sxoff + s * D : sxoff + (s + 1) * D],
                        rhs=oh[:],
                        start=False,
                        stop=(s == S - 1),
                        skip_group_check=True,
                    )
                s_bf = fp.tile([D, P], BF)
                nc.scalar.copy(out=s_bf[:], in_=acc[:])
                mlp = psb.tile([P, D], F32)
                nc.tensor.matmul(mlp[:], lhsT=s_bf[:], rhs=wf_sb[:], start=True, stop=True)
                if t % OB == 0:
                    outbs[t] = rp.tile([P, OB * D], BF, tag="outb", name=f"outb{t}")
                outb = outbs[t - t % OB]
                g = t % OB
                nc.scalar.activation(
                    out=outb[:, g * D : (g + 1) * D],
                    in_=mlp[:],
                    func=mybir.ActivationFunctionType.Copy,
                    scale=recip_sb[:, t : t + 1],
                )
                if t % OB == OB - 1 or t == nt - 1:
                    t0 = t - g
                    span = g + 1
                    nc.sync.dma_start(
                        out=out_d[:, t0 * D : (t0 + span) * D],
                        in_=outb[:, : span * D],
                    )

    return nc


def _prepare(inputs):
    """Host-side shard/layout prep.

    Returns (in_maps, meta) where meta = (node_of [NCORES, nt, P] global node
    id or -1, nt, S, bias [D], bias0 [D]).
    """
    from concourse import mybir

    bf16 = mybir.dt.np(mybir.dt.bfloat16)

    edge_attrs = np.asarray(inputs["edge_attrs"], dtype=np.float32)
    wp = np.asarray(inputs["proj_W"], dtype=np.float32)
    bp = np.asarray(inputs.get("proj_b", np.zeros(D)), dtype=np.float32)
    wu = np.asarray(inputs["upd_W"], dtype=np.float32)
    bu = np.asarray(inputs.get("upd_b", np.zeros(D)), dtype=np.float32)
    dst = np.asarray(inputs["dst"]).astype(np.int64).ravel()
    N = int(np.asarray(inputs["n_nodes"]))
    E = dst.shape[0]

    wf = np.ascontiguousarray(wp @ wu)
    bias = bp @ wu + bu  # added to nodes with >=1 edge
    bias0 = bu.copy()  # value for nodes with no edges

    perm = np.argsort(dst, kind="stable")
    sdst = dst[perm]
    sx = edge_attrs[perm]

    counts = np.bincount(sdst, minlength=N).astype(np.int64)
    cum = np.concatenate([[0], np.cumsum(counts)])
    rank = np.arange(E, dtype=np.int64) - cum[sdst]  # rank of edge within node
    recip_g = (1.0 / np.maximum(counts, 1)).astype(np.float32)

    npc = (N + NCORES - 1) // NCORES  # nodes per core
    nt = (npc + P - 1) // P

    # --- per-core planning: bin-pack nodes into tiles to balance spill ---
    core_plans = []
    max_spill_chunks = 1
    for k in range(NCORES):
        g0, g1 = k * npc, min((k + 1) * npc, N)
        nloc = g1 - g0
        cnt = counts[g0:g1]
        spill_n = np.maximum(cnt - C, 0)
        order = np.argsort(-spill_n, kind="stable")  # nodes sorted by spill desc
        # snake-deal across tiles to balance per-tile spill
        tile_of = np.empty(nloc, dtype=np.int64)
        col_of = np.empty(nloc, dtype=np.int64)
        pos = np.arange(nloc, dtype=np.int64)
        rnd = pos // nt  # round index
        tidx = pos % nt
        tidx = np.where(rnd % 2 == 1, nt - 1 - tidx, tidx)  # snake
        tile_of[order] = tidx
        col_of[order] = rnd
        tile_spill = np.bincount(tile_of, weights=spill_n, minlength=nt).astype(np.int64)
        need = int(np.ceil(tile_spill.max() / P)) if nloc else 1
        max_spill_chunks = max(max_spill_chunks, need, 1)
        core_plans.append((g0, g1, tile_of, col_of))
    S = max(2, max_spill_chunks)

    nchunk = nt * CH
    nspill = nt * S

    # constants shared by all cores
    a16 = (np.arange(P)[:, None] // C == np.arange(8)[None, :]).astype(bf16)
    iota = np.broadcast_to(np.arange(P, dtype=np.float32), (P, P)).astype(bf16)
    wf_bf = wf.astype(bf16)

    in_maps = []
    node_of_all = np.full((NCORES, nt, P), -1, dtype=np.int64)
    for k in range(NCORES):
        g0, g1, tile_of, col_of = core_plans[k]
        nloc = g1 - g0
        node_of_all[k, tile_of, col_of] = np.arange(g0, g1)

        # per-edge placement for this core's edges
        e0, e1 = int(cum[g0]), int(cum[g1])
        ed = sdst[e0:e1] - g0  # local node index per edge
        er = rank[e0:e1]
        et = tile_of[ed]
        ec = col_of[ed]

        x_base = np.zeros((nchunk * P, D), dtype=np.float32)
        base_m = er < C
        slot = (et * CH + ec // 8) * P + (ec % 8) * C + er
        x_base[slot[base_m]] = sx[e0:e1][base_m]

        x_spill = np.zeros((nspill * P, D), dtype=np.float32)
        sldst = np.full((nspill * P,), 1000.0, dtype=np.float32)
        sp_m = ~base_m
        sp_t = et[sp_m]
        # sequential slot within each tile's spill region
        order2 = np.argsort(sp_t, kind="stable")
        sp_rank = np.arange(sp_t.shape[0], dtype=np.int64)
        tile_start = np.searchsorted(sp_t[order2], np.arange(nt))
        sp_rank_sorted = sp_rank - tile_start[sp_t[order2]]
        assert sp_rank_sorted.max(initial=0) < S * P, "spill overflow; raise S"
        sslot = np.empty_like(sp_rank)
        sslot[order2] = (sp_t[order2] * S + sp_rank_sorted // P) * P + sp_rank_sorted % P
        x_spill[sslot] = sx[e0:e1][sp_m]
        sldst[sslot] = ec[sp_m]

        recip_dev = np.ones((P, nt), dtype=np.float32)
        recip_dev[col_of, tile_of] = recip_g[g0:g1]

        x_dev = np.ascontiguousarray(
            x_base.reshape(nchunk, P, D).transpose(1, 0, 2).reshape(P, nchunk * D)
        ).astype(bf16)
        sx_dev = np.ascontiguousarray(
            x_spill.reshape(nspill, P, D).transpose(1, 0, 2).reshape(P, nspill * D)
        ).astype(bf16)
        sldst_dev = np.ascontiguousarray(sldst.reshape(nspill, P).T)

        in_maps.append(
            {
                "x": x_dev,
                "sx": sx_dev,
                "sldst": sldst_dev,
                "wf": wf_bf,
                "a16": a16,
                "iota": iota,
                "recip": recip_dev,
            }
        )

    meta = (node_of_all, nt, S, bias, bias0, counts, N)
    return in_maps, meta


def _gather(results, meta):
    node_of_all, nt, S, bias, bias0, counts, N = meta
    out_full = np.zeros((N, D), dtype=np.float32)
    for k in range(NCORES):
        o = np.asarray(results[k]["out"], dtype=np.float32)  # [P, nt*D]
        o = o.reshape(P, nt, D).transpose(1, 0, 2)  # [nt, P, D]
        nid = node_of_all[k]  # [nt, P]
        m = nid >= 0
        out_full[nid[m]] = o[m]
    has_edge = counts > 0
    out_full[has_edge] += bias
    out_full[~has_edge] = bias0
    return out_full


def kernel(**inputs) -> np.ndarray:
    from concourse.bass_utils import run_bass_kernel_spmd

    in_maps, meta = _prepare(inputs)
    nt, S = meta[1], meta[2]
    nc = _build_program(nt, S)
    nc.finalize()
    res = run_bass_kernel_spmd(nc, in_maps, core_ids=list(range(NCORES)))
    return _gather(res.results, meta)


# revision 8
# speedup vs baseline: 1.8736x; 1.2150x over previous
"""GNN NodeBlock kernel for 8 TRN2 NeuronCores.

Math: out = (segment_mean(edge_attrs @ Wp + bp, dst)) @ Wu + bu.
Projection is linear, so it commutes with the segment sum: the two MLPs fuse
into one 64x64 weight Wf = Wp @ Wu applied to the per-node aggregate, and the
mean division (a per-node row scale) commutes with the feature matmul, so it
is applied after the update MLP. Biases reduce to a constant row added on the
host (zero for this problem's inputs).

Sharding: node-parallel, equal node ranges per core. Each core's nodes are
assigned to (tile, column) slots, 128 nodes per tile. Every node gets exactly
C=16 base edge slots, so a 128-slot chunk covers 8 consecutive columns with a
FIXED aggregation pattern A16[s, n] = (s//16 == n): one constant bf16 matrix,
no per-chunk onehot work. Each chunk is one matmul with the big X chunk as the
(free-to-load) stationary operand and A16 moving: 8 PSUM columns per chunk.

Edges beyond the 16 per-node slots ("spill", ~10% of edges) go through
data-dependent onehot chunks (DVE is_equal against an iota row), accumulated
into the same PSUM tile with start=False. The host concentrates spill-heavy
nodes into the early tiles (first-fit by spill count), so most tiles need no
spill chunks at all; the per-tile spill chunk counts s_list are baked into
the program (max across cores).

Per tile: 16 fixed matmuls + s_list[t] spill matmuls -> acc[64 feats, 128
nodes] in PSUM -> DVE copy to SBUF bf16 -> MLP matmul with Wf -> Act copy
with per-partition 1/count scale -> bf16 output in a [128, nt*64] column
layout (contiguous DMA descriptors). Host scatters rows back to node ids.
Engine queues: base X on SP, outputs on Act (issue deferred one tile so the
queue never blocks on compute), spill X + constants on gpsimd/SWDGE.
"""

import sys

sys.path.insert(0, "/opt/trn_rl_repo")

import numpy as np

P = 128
D = 64
NCORES = 8
C = 16  # base edge slots per node
CH = C  # base chunks per 128-node tile
SQB = 16  # spill chunks per spill DMA group
OB = 4  # tiles per output batch


def _x_groups(nt):
    """X DMA group starts: fours, then twos over the last ~8 tiles."""
    starts = []
    t = 0
    while t < nt:
        span = 4 if t + 8 <= nt else 2
        span = min(span, nt - t)
        starts.append((t, span))
        t += span
    return starts


def _build_program(nt, s_list):
    import concourse.bacc as bacc
    from concourse import mybir
    from concourse.tile import TileContext

    BF = mybir.dt.bfloat16
    F32 = mybir.dt.float32
    nchunk = nt * CH
    qstart = np.concatenate([[0], np.cumsum(s_list)]).astype(int)
    nq = int(qstart[-1])

    nc = bacc.Bacc(None, target_bir_lowering=False)
    x_d = nc.declare_dram_parameter("x", [P, nchunk * D], BF, isOutput=False)
    sx_d = nc.declare_dram_parameter("sx", [P, max(nq, 1) * D], BF, isOutput=False)
    sldst_d = nc.declare_dram_parameter("sldst", [P, max(nq, 1)], F32, isOutput=False)
    wf_d = nc.declare_dram_parameter("wf", [D, D], BF, isOutput=False)
    a16_d = nc.declare_dram_parameter("a16", [P, 8], BF, isOutput=False)
    iota_d = nc.declare_dram_parameter("iota", [P, P], BF, isOutput=False)
    recip_d = nc.declare_dram_parameter("recip", [P, nt], F32, isOutput=False)
    out_d = nc.declare_dram_parameter("out", [P, nt * D], BF, isOutput=True)

    xgroups = _x_groups(nt)
    tile_to_xg = {}
    for gi, (t0, span) in enumerate(xgroups):
        for t in range(t0, t0 + span):
            tile_to_xg[t] = (gi, t0, span)

    # spill DMA groups of SQB chunks; group g covers q in [g*SQB, (g+1)*SQB)
    nsg = (nq + SQB - 1) // SQB
    # first tile that uses each spill group
    first_tile = {}
    for t in range(nt):
        for q in range(qstart[t], qstart[t + 1]):
            g = q // SQB
            if g not in first_tile:
                first_tile[g] = t
    # emit each group's DMA 4 tiles before first use (clamped)
    sched = {}
    for g in range(nsg):
        et = max(0, first_tile.get(g, 0) - 4)
        sched.setdefault(et, []).append(g)

    with TileContext(nc) as tc:
        with (
            tc.tile_pool(name="const", bufs=1) as cp,
            tc.tile_pool(name="xin", bufs=3) as xp,
            tc.tile_pool(name="sxin", bufs=2) as sxp,
            tc.tile_pool(name="oh", bufs=8) as ohp,
            tc.tile_pool(name="sbf", bufs=3) as fp,
            tc.tile_pool(name="res", bufs=2) as rp,
            tc.tile_pool(name="psacc", bufs=3, space="PSUM") as psa,
            tc.tile_pool(name="psmlp", bufs=2, space="PSUM") as psb,
        ):
            wf_sb = cp.tile([D, D], BF)
            nc.gpsimd.dma_start(out=wf_sb[:], in_=wf_d[:])
            a16_sb = cp.tile([P, 8], BF)
            nc.gpsimd.dma_start(out=a16_sb[:], in_=a16_d[:])
            iota_sb = cp.tile([P, P], BF)
            nc.gpsimd.dma_start(out=iota_sb[:], in_=iota_d[:])
            recip_sb = cp.tile([P, nt], F32)
            nc.gpsimd.dma_start(out=recip_sb[:], in_=recip_d[:])
            sldst_sb = cp.tile([P, max(nq, 1)], F32)
            nc.gpsimd.dma_start(out=sldst_sb[:], in_=sldst_d[:])

            xins = {}
            sxins = {}
            outbs = {}
            ohs = {}
            s_bfs = {}
            pending_out = []

            def emit_spill_groups(t):
                for g in sched.get(t, []):
                    q0 = g * SQB
                    span = min(SQB, nq - q0)
                    sxin = sxp.tile([P, SQB * D], BF, tag="sxin", name=f"sxin{g}")
                    nc.gpsimd.dma_start(
                        out=sxin[:, : span * D],
                        in_=sx_d[:, q0 * D : (q0 + span) * D],
                    )
                    sxins[g] = sxin

            def emit_onehots(t):
                pair = []
                for s in range(s_list[t]):
                    q = int(qstart[t]) + s
                    oh = ohp.tile([P, P], BF)
                    nc.vector.tensor_scalar(
                        out=oh[:],
                        in0=iota_sb[:],
                        scalar1=sldst_sb[:, q : q + 1],
                        scalar2=None,
                        op0=mybir.AluOpType.is_equal,
                    )
                    pair.append((q, oh))
                ohs[t] = pair

            def emit_tail(t):
                """MLP + scale for tile t; flush the PREVIOUS output group's DMA."""
                mlp = psb.tile([P, D], F32)
                nc.tensor.matmul(
                    mlp[:], lhsT=s_bfs.pop(t)[:], rhs=wf_sb[:], start=True, stop=True
                )
                if t % OB == 0:
                    outbs[t] = rp.tile([P, OB * D], BF, tag="outb", name=f"outb{t}")
                outb = outbs[t - t % OB]
                g = t % OB
                nc.scalar.activation(
                    out=outb[:, g * D : (g + 1) * D],
                    in_=mlp[:],
                    func=mybir.ActivationFunctionType.Copy,
                    scale=recip_sb[:, t : t + 1],
                )
                if t % OB == OB - 1 or t == nt - 1:
                    t0 = t - g
                    pending_out.append((t0, g + 1))
                # flush output DMA for the group completed BEFORE this one:
                # its data has been ready for a while, so the Act queue never
                # stalls waiting for compute.
                while len(pending_out) > 1:
                    f0, fspan = pending_out.pop(0)
                    nc.scalar.dma_start(
                        out=out_d[:, f0 * D : (f0 + fspan) * D],
                        in_=outbs[f0][:, : fspan * D],
                    )

            emit_spill_groups(0)
            emit_onehots(0)
            for t in range(nt):
                if t in tile_to_xg and tile_to_xg[t][1] == t:
                    gi, t0, span = tile_to_xg[t]
                    xin = xp.tile([P, 4 * CH * D], BF, tag="xin", name=f"xin{t}")
                    nc.sync.dma_start(
                        out=xin[:, : span * CH * D],
                        in_=x_d[:, t * CH * D : (t + span) * CH * D],
                    )
                    xins[gi] = xin
                if t > 0:
                    emit_spill_groups(t)
                gi, t0, span = tile_to_xg[t]
                xin = xins[gi]
                xoff = (t - t0) * CH * D

                if t + 1 < nt:
                    emit_onehots(t + 1)

                ns = s_list[t]
                acc = psa.tile([D, P], F32)
                for j in range(CH):
                    nc.tensor.matmul(
                        acc[:, 8 * j : 8 * (j + 1)],
                        lhsT=xin[:, xoff + j * D : xoff + (j + 1) * D],
                        rhs=a16_sb[:],
                        start=(j == 0),
                        stop=(ns == 0 and j == CH - 1),
                        skip_group_check=True,
                    )
                for s in range(ns):
                    q, oh = ohs[t][s]
                    sg = q // SQB
                    nc.tensor.matmul(
                        acc[:],
                        lhsT=sxins[sg][:, (q - sg * SQB) * D : (q - sg * SQB + 1) * D],
                        rhs=oh[:],
                        start=False,
                        stop=(s == ns - 1),
                        skip_group_check=True,
                    )
                del ohs[t]
                s_bf = fp.tile([D, P], BF)
                nc.vector.tensor_copy(s_bf[:], acc[:])
                s_bfs[t] = s_bf
                if t > 0:
                    emit_tail(t - 1)
            emit_tail(nt - 1)
            for f0, fspan in pending_out:
                nc.scalar.dma_start(
                    out=out_d[:, f0 * D : (f0 + fspan) * D],
                    in_=outbs[f0][:, : fspan * D],
                )

    return nc


def _prepare(inputs):
    """Host-side shard/layout prep.

    Returns (in_maps, meta); meta = (node_of [NCORES, nt, P], nt, s_list,
    bias, bias0, counts, N).
    """
    from concourse import mybir

    bf16 = mybir.dt.np(mybir.dt.bfloat16)

    edge_attrs = np.asarray(inputs["edge_attrs"], dtype=np.float32)
    wp = np.asarray(inputs["proj_W"], dtype=np.float32)
    bp = np.asarray(inputs.get("proj_b", np.zeros(D)), dtype=np.float32)
    wu = np.asarray(inputs["upd_W"], dtype=np.float32)
    bu = np.asarray(inputs.get("upd_b", np.zeros(D)), dtype=np.float32)
    dst = np.asarray(inputs["dst"]).astype(np.int64).ravel()
    N = int(np.asarray(inputs["n_nodes"]))
    E = dst.shape[0]

    wf = np.ascontiguousarray(wp @ wu)
    bias = bp @ wu + bu  # added to nodes with >=1 edge
    bias0 = bu.copy()  # value for nodes with no edges

    perm = np.argsort(dst, kind="stable")
    sdst = dst[perm]
    sx = edge_attrs[perm]

    counts = np.bincount(sdst, minlength=N).astype(np.int64)
    cum = np.concatenate([[0], np.cumsum(counts)])
    rank = np.arange(E, dtype=np.int64) - cum[sdst]
    recip_g = (1.0 / np.maximum(counts, 1)).astype(np.float32)

    npc = (N + NCORES - 1) // NCORES
    nt = (npc + P - 1) // P

    # --- per-core planning: concentrate spill-heavy nodes into early tiles ---
    core_plans = []
    s_need = np.zeros((NCORES, nt), dtype=np.int64)
    for k in range(NCORES):
        g0, g1 = k * npc, min((k + 1) * npc, N)
        nloc = g1 - g0
        cnt = counts[g0:g1]
        spill_n = np.maximum(cnt - C, 0)
        cap = max(2 * P, int(spill_n.max(initial=0)))
        order = np.argsort(-spill_n, kind="stable")
        tile_of = np.empty(nloc, dtype=np.int64)
        col_of = np.empty(nloc, dtype=np.int64)
        used = np.zeros(nt, dtype=np.int64)
        tspill = np.zeros(nt, dtype=np.int64)
        ti = 0
        nnz = int((spill_n > 0).sum())
        for n in order[:nnz]:
            sp = spill_n[n]
            while used[ti] == P or tspill[ti] + sp > cap:
                ti += 1
            tile_of[n] = ti
            col_of[n] = used[ti]
            used[ti] += 1
            tspill[ti] += sp
        # fill remaining columns with zero-spill nodes, tile-major
        free = P - used
        ztiles = np.repeat(np.arange(nt), free)
        zcols = np.concatenate([np.arange(used[t], P) for t in range(nt)]) if nt else np.array([], dtype=np.int64)
        zn = order[nnz:]
        tile_of[zn] = ztiles[: zn.shape[0]]
        col_of[zn] = zcols[: zn.shape[0]]
        s_need[k] = (tspill + P - 1) // P
        core_plans.append((g0, g1, tile_of, col_of))
    s_list = s_need.max(axis=0).astype(int).tolist()
    qstart = np.concatenate([[0], np.cumsum(s_list)]).astype(np.int64)
    nq = int(qstart[-1])

    nchunk = nt * CH
    a16 = (np.arange(P)[:, None] // C == np.arange(8)[None, :]).astype(bf16)
    iota = np.broadcast_to(np.arange(P, dtype=np.float32), (P, P)).astype(bf16)
    wf_bf = wf.astype(bf16)

    in_maps = []
    node_of_all = np.full((NCORES, nt, P), -1, dtype=np.int64)
    for k in range(NCORES):
        g0, g1, tile_of, col_of = core_plans[k]
        node_of_all[k, tile_of, col_of] = np.arange(g0, g1)

        e0, e1 = int(cum[g0]), int(cum[g1])
        ed = sdst[e0:e1] - g0
        er = rank[e0:e1]
        et = tile_of[ed]
        ec = col_of[ed]

        x_base = np.zeros((nchunk * P, D), dtype=np.float32)
        base_m = er < C
        slot = (et * CH + ec // 8) * P + (ec % 8) * C + er
        x_base[slot[base_m]] = sx[e0:e1][base_m]

        x_spill = np.zeros((max(nq, 1) * P, D), dtype=np.float32)
        sldst = np.full((max(nq, 1) * P,), 1000.0, dtype=np.float32)
        sp_m = ~base_m
        sp_t = et[sp_m]
        order2 = np.argsort(sp_t, kind="stable")
        sp_rank = np.arange(sp_t.shape[0], dtype=np.int64)
        tile_start = np.searchsorted(sp_t[order2], np.arange(nt))
        sp_rank_sorted = sp_rank - tile_start[sp_t[order2]]
        sslot = np.empty_like(sp_rank)
        sslot[order2] = (qstart[sp_t[order2]] + sp_rank_sorted // P) * P + sp_rank_sorted % P
        assert sp_rank_sorted.max(initial=0) < np.asarray(s_list)[sp_t[order2]].max(initial=1) * P
        x_spill[sslot] = sx[e0:e1][sp_m]
        sldst[sslot] = ec[sp_m]

        recip_dev = np.ones((P, nt), dtype=np.float32)
        recip_dev[col_of, tile_of] = recip_g[g0:g1]

        x_dev = np.ascontiguousarray(
            x_base.reshape(nchunk, P, D).transpose(1, 0, 2).reshape(P, nchunk * D)
        ).astype(bf16)
        sx_dev = np.ascontiguousarray(
            x_spill.reshape(max(nq, 1), P, D).transpose(1, 0, 2).reshape(P, max(nq, 1) * D)
        ).astype(bf16)
        sldst_dev = np.ascontiguousarray(sldst.reshape(max(nq, 1), P).T)

        in_maps.append(
            {
                "x": x_dev,
                "sx": sx_dev,
                "sldst": sldst_dev,
                "wf": wf_bf,
                "a16": a16,
                "iota": iota,
                "recip": recip_dev,
            }
        )

    meta = (node_of_all, nt, s_list, bias, bias0, counts, N)
    return in_maps, meta


def _gather(results, meta):
    node_of_all, nt, s_list, bias, bias0, counts, N = meta
    out_full = np.zeros((N, D), dtype=np.float32)
    for k in range(NCORES):
        o = np.asarray(results[k]["out"], dtype=np.float32)  # [P, nt*D]
        o = o.reshape(P, nt, D).transpose(1, 0, 2)  # [nt, P, D]
        nid = node_of_all[k]
        m = nid >= 0
        out_full[nid[m]] = o[m]
    has_edge = counts > 0
    out_full[has_edge] += bias
    out_full[~has_edge] = bias0
    return out_full


def kernel(**inputs) -> np.ndarray:
    from concourse.bass_utils import run_bass_kernel_spmd

    in_maps, meta = _prepare(inputs)
    nt, s_list = meta[1], meta[2]
    nc = _build_program(nt, s_list)
    nc.finalize()
    res = run_bass_kernel_spmd(nc, in_maps, core_ids=list(range(NCORES)))
    return _gather(res.results, meta)
